# revision 1
# baseline (speedup 1.0000x reference)
"""TRN2 Bass kernel for nn_Attention_47665547051353.

Reference computation (B=4, C=512, N=2048, H=8, hd=64):
    qkv  = w_qkv @ x                           # 1x1 conv
    attn = softmax_j( k^T q * hd^-0.5 )        # softmax over QUERIES j
    out  = w_proj @ (v @ attn) + b_proj

Sharding (8 cores): core c -> batch b = c//2, head-group g = c%2 (4 heads).
Each core computes its heads' full attention plus a partial output
projection; the host sums the two partial projections per batch and adds
the bias.

Design (measured on HW, ~219-260us/core depending on device state):
  - Everything runs in fp16 on the PE (full rate for half-array shapes;
    fp32r is 2x slower at K=64/M=64) with fp32 PSUM accumulation; inputs
    are pre-cast to fp16 on the host so no on-device rounding pass exists.
  - Softmax skips max-subtraction (scores are ~N(0,1) by construction;
    exp is safe in fp32).  The per-key normalizer 1/sum_j exp(s_ij) is
    folded into v, the contraction operand of the v @ attn matmul.
  - The exp stream on the Scalar engine is the roofline (~128 x 1.2us);
    the schedule keeps it saturated: scores double-buffered in PSUM
    (2x[128,1024] = 4 banks) + per-head AV accumulator (4 banks).
  - Consecutive matmuls alternate PE row groups (via swapped-half copies
    of K/Q) and AV output col groups (via (i+jc)-parity tile_position)
    so each LDWEIGHTS overlaps the previous matmul; the AV partition
    halves are summed for free by duplicated w_proj rows.
  - V^T comes from 32 fp16 transpose-DMAs instead of PE matmuls; pair-1
    QKV projections are interleaved into the first attention units' PE
    slack (software pipelining with a pending-AV queue).
"""
import sys

if "/opt/trn_rl_repo" not in sys.path:
    sys.path.insert(0, "/opt/trn_rl_repo")

import numpy as np

import concourse.bass as bass
import concourse.tile as tile
import concourse.mybir as mybir
from concourse import bacc
from concourse.bass_utils import run_bass_kernel_spmd

F32 = mybir.dt.float32
F32R = mybir.dt.float32r
F16 = mybir.dt.float16
U16 = mybir.dt.uint16
EXP = mybir.ActivationFunctionType.Exp
MULT = mybir.AluOpType.mult
ADD = mybir.AluOpType.add

B, C, N = 4, 512, 2048
H, HD = 8, 64
SCALE = HD ** -0.5
P = 128
CC = C // P          # 4 contraction chunks over channels
NT = N // P          # 16 key blocks
HG = H // 2          # 4 heads per core (one head-group)
N_CORES = 8

# Schraudolph fp16 exp: u16 bits = round(score * TS_A + TS_B); max rel
# err ~3%, the softmax normalizer cancels the mean component.
LOG2E = 1.4426950408889634
TS_A = SCALE * LOG2E * 1024.0
TS_B = 15360.0 - 45.0

# Units (0..63, head-major) whose exp runs on the Vector engine instead
# of ACT.  Only legal for units 8..47: the 3-buffer score rotation starts
# after the prologue's 2-buffer phase and ends when the last head
# reclaims the third buffer's banks for its full-width AV accumulator.
# With 3 buffers the ACT stream never waits on the DVE drain.
OFFLOAD = frozenset(
    {9, 12} | {16 * hl + i for hl in (1, 2) for i in (5, 8, 11, 14)})

_CACHE = {}


def build_program(dbg=False, phases=("qkv", "vt", "attn", "proj"),
                  attn_tb=((0, 0), (0, 1), (1, 0), (1, 1)),
                  offload=OFFLOAD):
    nc = bacc.Bacc("TRN2", target_bir_lowering=False, debug=False)
    x_ap = nc.dram_tensor("x", [C, N], F16, kind="ExternalInput").ap()
    wq_ap = nc.dram_tensor("wqT", [C, HG * HD], F16, kind="ExternalInput").ap()
    wk_ap = nc.dram_tensor("wkT", [C, HG * HD], F16, kind="ExternalInput").ap()
    wv_ap = nc.dram_tensor("wvT", [C, HG * HD], F16, kind="ExternalInput").ap()
    wp_ap = nc.dram_tensor("wpT", [HG * P, C], F16, kind="ExternalInput").ap()
    out_ap = nc.dram_tensor("out", [C, N], F32, kind="ExternalOutput").ap()

    with tile.TileContext(nc) as tc:
        with (
            tc.tile_pool(name="const", bufs=1) as const,
            tc.tile_pool(name="big", bufs=1) as big,
            tc.tile_pool(name="ppool", bufs=22) as ppool,
            tc.tile_pool(name="small", bufs=16) as small,
            tc.tile_pool(name="vpp", bufs=24) as vpp,
            tc.tile_pool(name="outp", bufs=2) as outp,
        ):
            # ACT exp-table preload (overlaps the input DMAs)
            warm = small.tile([P, 1], F32, tag="warm")
            warm2 = small.tile([P, 1], F32, tag="warm2")
            nc.vector.memset(warm, 0.0)
            nc.scalar.activation(warm2, warm, EXP)

            # scores pool first so it owns banks not shared with the
            # prologue pool (attention can start mid-prologue)
            scps_cm = tc.tile_pool(name="scps", bufs=2, space="PSUM")
            scps = scps_cm.__enter__()

            # PE pre-warm: ~32 discarded zero-input matmuls spanning the
            # input-DMA wire phase, so the PE reaches its full p-state
            # (2.4GHz needs ~3us of continuous work) before x lands and
            # the first QKV chain runs at 215ns/matmul instead of ~630.
            wz = const.tile([P, HD], F16, tag="wz")
            wzm = const.tile([P, 512], F16, tag="wzm")
            nc.vector.memset(wz, 0.0)
            nc.vector.memset(wzm, 0.0)
            # the PE's first instruction clears the framework boot
            # barrier at ~11.8us and x lands ~13.5us: 5 warm matmuls ramp
            # the p-state and end just as the real QKV chain's data
            # arrives (32 of them delayed q00 by ~4us, measured)
            for w in range(5):
                wps = scps.tile([P, 1024], F32, tag="s", name=f"warmm{w}")
                nc.tensor.matmul(wps[0:HD, 0:512], wz, wzm,
                                 start=True, stop=True)

            QK = {}
            VT = big.tile([P, NT, HG * HD], F16)
            wp_r = const.tile([P, 4, C], F16)
            scr = big.tile([P, N], F16, tag="sumscr")
            A = {}
            units = [(t, h, i) for t in range(2) for h in range(2)
                     if (t, h) in attn_tb for i in range(NT)]
            av_tiles = {}
            headbuf = {}   # (t,h) -> [(i, vp, p_t)] for the AV right pass
            pending = []
            # score-buffer rotation: 2 buffers (scps) during the prologue,
            # 3 buffers (scps + scps2) once the prologue PSUM pool retires.
            # With 3 buffers an offloaded unit's DVE drain never blocks the
            # ACT stream: the next ACT half reuses a buffer last drained by
            # ACT itself.
            rot = {"pools": [scps, scps], "n": 0}

            def score_tile():
                pools = rot["pools"]
                p = pools[rot["n"] % len(pools)]
                rot["n"] += 1
                return p.tile([P, 1024], F32, tag="s", name=f"sc{rot['n']}")

            def emit_unit(t, h, i, use_dve=False):
                kt, qt = QK[("k", t)], QK[("q", t)]
                ktd, qtd = QK.get(("kd", t)), QK.get(("qd", t))
                p_t = ppool.tile([P, N], F16, tag="p")
                sv = []
                first_units = (t == 0 and h == 0 and i < 8)
                for half in range(2):
                    sps = score_tile()
                    for jc in range(2):
                        # alternate PE row groups per matmul so each
                        # LDWEIGHTS overlaps the previous matmul (units 0-7
                        # skip it: the swapped duplicates aren't DMA'd yet
                        # and would stall the in-order PE)
                        if (i + jc) % 2 == 0 or first_units:
                            kk, qq, rb = kt, qt, h * HD
                        else:
                            kk, qq, rb = ktd, qtd, (1 - h) * HD
                        nc.tensor.matmul(
                            sps[:, jc * 512:(jc + 1) * 512],
                            kk[rb:rb + HD, i * P:(i + 1) * P],
                            qq[rb:rb + HD,
                               half * 1024 + jc * 512:half * 1024 + (jc + 1) * 512],
                            start=True, stop=True,
                        )
                    if use_dve:
                        # Schraudolph exp bits straight into the f16 p tile
                        # (doubles as the PSUM->SBUF move)
                        nc.vector.tensor_scalar(
                            p_t[:, half * 1024:(half + 1) * 1024].bitcast(U16),
                            sps, TS_A, TS_B, MULT, ADD)
                    else:
                        s_t = small.tile([P, 1], F32, tag=f"sum{half}")
                        sv.append(s_t)
                        nc.scalar.activation(
                            p_t[:, half * 1024:(half + 1) * 1024], sps,
                            EXP, scale=SCALE, accum_out=s_t)
                s_all = small.tile([P, 1], F32, tag="stot")
                if use_dve:
                    nc.vector.tensor_scalar(
                        scr, p_t, 1.0, None, MULT, ADD, accum_out=s_all)
                else:
                    nc.vector.tensor_add(s_all, sv[0], sv[1])
                r_t = small.tile([P, 1], F32, tag="rcp")
                nc.vector.reciprocal(r_t, s_all)
                return p_t, r_t

            def av_matmul(av, vp, p_t, i, jc4, wide=False):
                # alternate output col groups per matmul; the parity halves
                # are summed by the duplicated projection rows
                par = (i + jc4) % 2
                q0 = (par + jc4) % 2
                oc = jc4 if wide else (jc4 % 2)
                nc.tensor.matmul(
                    av[par * HD:(par + 1) * HD, oc * 512:oc * 512 + 512],
                    vp,
                    p_t[:, jc4 * 512:(jc4 + 1) * 512],
                    start=(i == q0), stop=(i == NT - 2 + q0),
                    tile_position=(0, par * HD),
                    skip_group_check=True,
                )

            def emit_av(avps, vpp, t, h, i, p_t, r_t, full=False):
                """AV accumulation: left half (query cols 0-1023) for split
                heads, or all 2048 query cols when full=True (the last
                head, whose accumulator takes over scps2+avps banks so no
                right-pass replay trails into the projection)."""
                vp = vpp.tile([P, HD], F16, tag="vp")
                hl = 2 * t + h
                nc.vector.tensor_scalar_mul(
                    vp, VT[:, i, hl * HD:(hl + 1) * HD], r_t)
                if (t, h) not in av_tiles:
                    cols = N if full else 1024
                    av_tiles[(t, h)] = avps.tile(
                        [P, cols], F32, tag="av", name=f"avl{2*t+h}")
                    headbuf[(t, h)] = []
                av = av_tiles[(t, h)]
                for jc4 in ((0, 1, 2, 3) if full else (0, 1)):
                    av_matmul(av, vp, p_t, i, jc4, wide=full)
                headbuf[(t, h)].append((i, vp, p_t))
                if i == NT - 1:
                    a_h = big.tile([P, N], F16, tag=f"a{t}{h}")
                    av_done = av_tiles.pop((t, h))
                    for q4 in range(4 if full else 2):
                        nc.vector.tensor_copy(
                            a_h[:, q4 * 512:(q4 + 1) * 512],
                            av_done[:, q4 * 512:(q4 + 1) * 512])
                    A[(t, h)] = a_h

            def emit_av_r(avps, t, h, blocks):
                """Right half (query cols 1024-2047), replayed from the
                persisted vp/p tiles once the left accumulator's banks have
                been freed."""
                key = ("r", t, h)
                if key not in av_tiles:
                    av_tiles[key] = avps.tile(
                        [P, 1024], F32, tag="av", name=f"avr{2*t+h}")
                av = av_tiles[key]
                buf = headbuf[(t, h)]
                for b in blocks:
                    i, vp, p_t = buf[b]
                    for jc4 in (2, 3):
                        av_matmul(av, vp, p_t, i, jc4)
                if blocks[-1] == NT - 1:
                    a_h = A[(t, h)]
                    av_done = av_tiles.pop(key)
                    for q4 in range(2):
                        nc.vector.tensor_copy(
                            a_h[:, 1024 + q4 * 512:1024 + (q4 + 1) * 512],
                            av_done[:, q4 * 512:(q4 + 1) * 512])
                    del headbuf[(t, h)]

            with tc.tile_pool(name="ld", bufs=1) as ld, \
                 tc.tile_pool(name="props", bufs=2, space="PSUM") as props:
                # ---- loads + fp32r rounding (DVE/GPSIMD in parallel) ----
                # all inputs arrive pre-cast to fp16 from the host;
                # x on the sync queue (gates QK0), weights on gpsimd's
                x_r = ld.tile([P, CC, N], F16)
                x_view = x_ap.rearrange("(cc p) n -> cc p n", p=P)
                for cc in range(CC):
                    nc.sync.dma_start(out=x_r[:, cc, :], in_=x_view[cc])
                wq_r = ld.tile([P, CC, HG * HD], F16)
                wk_r = ld.tile([P, CC, HG * HD], F16)
                wv_r = ld.tile([P, CC, HG * HD], F16)
                nc.gpsimd.dma_start(out=wq_r, in_=wq_ap.rearrange("(cc p) o -> p cc o", p=P))
                nc.gpsimd.dma_start(out=wk_r, in_=wk_ap.rearrange("(cc p) o -> p cc o", p=P))
                nc.gpsimd.dma_start(out=wv_r, in_=wv_ap.rearrange("(cc p) o -> p cc o", p=P))
                nc.gpsimd.dma_start(out=wp_r, in_=wp_ap.rearrange("(t p) o -> p t o", p=P))

                def emit_qk_chunk(wname, w_r, t, half):
                    """One [128,1024] output chunk of a q/k projection."""
                    key = (wname, t)
                    if key not in QK:
                        dst_new = big.tile([P, N], F16, tag=f"{wname}{t}")
                        QK[key] = dst_new
                    dst = QK[key]
                    ps = props.tile([P, 1024], F32, tag="qk")
                    for cc in range(CC):
                        for jc in range(2):
                            j0 = jc * 512
                            nc.tensor.matmul(
                                ps[:, j0:j0 + 512],
                                w_r[:, cc, t * P:(t + 1) * P],
                                x_r[:, cc, half * 1024 + j0:half * 1024 + j0 + 512],
                                start=(cc == 0), stop=(cc == CC - 1),
                            )
                    nc.vector.tensor_copy(dst[:, half * 1024:(half + 1) * 1024], ps)
                    if half == 1:
                        dstd = big.tile([P, N], F16, tag=f"{wname}d{t}")
                        nc.sync.dma_start(out=dstd[0:HD, :], in_=dst[HD:2 * HD, :])
                        nc.sync.dma_start(out=dstd[HD:2 * HD, :], in_=dst[0:HD, :])
                        QK[(wname + "d", t)] = dstd

                # pair-0 Q/K first so attention can start ASAP; units
                # 0-7 only read k columns < 1024, so k's second half is
                # deferred into the fill list below
                emit_qk_chunk("q", wq_r, 0, 0)
                emit_qk_chunk("q", wq_r, 0, 1)
                emit_qk_chunk("k", wk_r, 0, 0)

                def emit_v_chunk(vt2, half, vr):
                    ps = props.tile([P, 1024], F32, tag="qk")
                    for cc in range(CC):
                        for jc in range(2):
                            j0 = jc * 512
                            nc.tensor.matmul(
                                ps[:, j0:j0 + 512],
                                wv_r[:, cc, vt2 * P:(vt2 + 1) * P],
                                x_r[:, cc, half * 1024 + j0:half * 1024 + j0 + 512],
                                start=(cc == 0), stop=(cc == CC - 1),
                            )
                    nc.vector.tensor_copy(vr[:, half * 1024:(half + 1) * 1024], ps)
                    if half == 1:
                        for nt in range(NT):
                            nc.sync.dma_start(
                                out=VT[:, nt, vt2 * P:(vt2 + 1) * P],
                                in_=vr[:, nt * P:(nt + 1) * P],
                                transpose=True,
                            )

                # v projections / VT transposes / pair-1 Q/K interleave
                # into the first attention units' PE slack
                vrow0 = ld.tile([P, N], F16, tag="vrow0")
                vrow1 = ld.tile([P, N], F16, tag="vrow1")
                vrow = [vrow0, vrow1]
                fill = [lambda: emit_qk_chunk("k", wk_r, 0, 1),
                        lambda: emit_v_chunk(0, 0, vrow[0]),
                        lambda: emit_v_chunk(0, 1, vrow[0]),
                        lambda: emit_v_chunk(1, 0, vrow[1]),
                        lambda: emit_v_chunk(1, 1, vrow[1]),
                        lambda: emit_qk_chunk("q", wq_r, 1, 0),
                        lambda: emit_qk_chunk("q", wq_r, 1, 1),
                        lambda: emit_qk_chunk("k", wk_r, 1, 0),
                        lambda: emit_qk_chunk("k", wk_r, 1, 1)]
                n_pre = min(8, len(units)) if ("attn" in phases) else 0
                for g in range(n_pre):
                    u = units[g]
                    pending.append((u, emit_unit(*u)))
                    if g < len(fill):
                        fill[g]()
                for f in fill[n_pre:]:
                    f()

            # third score buffer: its banks come from the retired prologue
            # PSUM pool (prologue: scps 4 + props 4; stream: scps 4 +
            # scps2 2 + avps 2)
            scps2_cm = tc.tile_pool(name="scps2", bufs=1, space="PSUM")
            scps2 = scps2_cm.__enter__()
            rot["pools"] = [scps, scps, scps2]

            # ---- main attention stream (software-pipelined) ----
            head_order = [u[:2] for u in units[::NT]]
            last_head = head_order[-1]
            avps_cm = tc.tile_pool(name="avps", bufs=1, space="PSUM")
            avps = avps_cm.__enter__()
            av2ps = None
            if "attn" in phases:
                for g in range(n_pre, len(units)):
                    u = units[g]
                    phase_i = g % NT
                    in_last = (g // NT == len(head_order) - 1)
                    if in_last and phase_i == 0:
                        # the last head streams on the 2-buffer rotation;
                        # scps2's banks go to its full-width accumulator
                        rot["pools"] = [scps, scps]
                    # replay the previous head's right-half AV from the
                    # persisted p/vp tiles (its av_L banks are free now)
                    if g >= NT and phase_i < 4:
                        pt_prev = head_order[g // NT - 1]
                        emit_av_r(avps, *pt_prev,
                                  blocks=range(4 * phase_i, 4 * phase_i + 4))
                    if in_last and phase_i == 4:
                        # scps2 drained (last tile was unit 47's) and the
                        # final av_R copies are emitted: hand both pools'
                        # banks to the last head's [128, 2048] accumulator
                        avps_cm.__exit__(None, None, None)
                        scps2_cm.__exit__(None, None, None)
                        av2_cm = tc.tile_pool(name="av2ps", bufs=1,
                                              space="PSUM")
                        av2ps = av2_cm.__enter__()
                    use_dve = (g in offload) and not in_last
                    pending.append((u, emit_unit(*u, use_dve=use_dve)))
                    # steady AV lag of 5 units keeps the in-order PE from
                    # stalling QK chains behind not-yet-ready AVs (and
                    # gives the DVE queue slack to finish av_R copies
                    # before av_L's allocation); taper to zero at head end
                    # so av_L's banks free before the next head's replay
                    if phase_i == NT - 1:
                        drain_to = 0
                    elif phase_i >= NT - 3:
                        drain_to = NT - 1 - phase_i
                    else:
                        drain_to = max(5, 9 - max(0, g - n_pre + 1))
                    while len(pending) > drain_to:
                        (pt_, ph_, pi_), (p_t, r_t) = pending.pop(0)
                        full = (pt_, ph_) == last_head
                        emit_av(av2ps if full else avps, vpp,
                                pt_, ph_, pi_, p_t, r_t, full=full)
                while pending:
                    (pt_, ph_, pi_), (p_t, r_t) = pending.pop(0)
                    full = (pt_, ph_) == last_head
                    emit_av(av2ps if full else avps, vpp,
                            pt_, ph_, pi_, p_t, r_t, full=full)
                av2_cm.__exit__(None, None, None)
            else:
                avps_cm.__exit__(None, None, None)

            scps_cm.__exit__(None, None, None)

            # ---- output projection (fp16, duplicated-row weight chunks) ----
            with tc.tile_pool(name="prps", bufs=2, space="PSUM") as prps:
              if "proj" in phases and len(A) == 4:
                for ot in range(4):
                    pso = prps.tile([P, N], F32)
                    for jc in range(4):
                        for hi in range(4):
                            t2, h2 = hi // 2, hi % 2
                            nc.tensor.matmul(
                                pso[:, jc * 512:(jc + 1) * 512],
                                wp_r[:, hi, ot * P:(ot + 1) * P],
                                A[(t2, h2)][:, jc * 512:(jc + 1) * 512],
                                start=(hi == 0), stop=(hi == 3),
                            )
                    o_sb = outp.tile([P, N], F32, tag="o")
                    nc.vector.tensor_copy(o_sb, pso)
                    nc.sync.dma_start(out=out_ap[ot * P:(ot + 1) * P, :], in_=o_sb)

    nc.compile()
    return nc


def _shard_weights(w_qkv, w_proj):
    """Per head-group g: transposed q/k/v weight shards [C, 256] with output
    column order o = 64*h_local + d, and projection shard [256, C]."""
    shards = []
    for g in range(2):
        heads = range(HG * g, HG * (g + 1))
        q_rows = [h * 3 * HD + d for h in heads for d in range(HD)]
        k_rows = [h * 3 * HD + HD + d for h in heads for d in range(HD)]
        v_rows = [h * 3 * HD + 2 * HD + d for h in heads for d in range(HD)]
        a_chans = [h * HD + (r % HD) for h in heads for r in range(P)]
        shards.append({
            "wqT": np.ascontiguousarray(w_qkv[q_rows, :].T),
            "wkT": np.ascontiguousarray(w_qkv[k_rows, :].T),
            "wvT": np.ascontiguousarray(w_qkv[v_rows, :].T),
            "wpT": np.ascontiguousarray(w_proj[:, a_chans].T),
        })
    return shards


def kernel(x, w_qkv, w_proj, b_proj, _trace=False, _trace_kwargs=None):
    x = np.asarray(x, dtype=np.float32)
    w_qkv = np.asarray(w_qkv, dtype=np.float32)
    w_proj = np.asarray(w_proj, dtype=np.float32)
    b_proj = np.asarray(b_proj, dtype=np.float32)

    if "nc" not in _CACHE:
        _CACHE["nc"] = build_program()
    nc = _CACHE["nc"]

    shards = _shard_weights(w_qkv, w_proj)
    shards = [{k: v.astype(np.float16) for k, v in s.items()} for s in shards]
    in_maps = []
    for core in range(N_CORES):
        b, g = core // 2, core % 2
        m = {"x": np.ascontiguousarray(x[b].astype(np.float16))}
        m.update(shards[g])
        in_maps.append(m)

    kw = {}
    if _trace:
        kw.update(trace=True, trace_cores=[0], **(_trace_kwargs or {}))
    res = run_bass_kernel_spmd(nc, in_maps, list(range(N_CORES)), **kw)

    out = np.empty((B, C, N), dtype=np.float32)
    for b in range(B):
        out[b] = (res.results[2 * b]["out"] + res.results[2 * b + 1]["out"]
                  + b_proj[:, None])
    if _trace:
        _CACHE["last_result"] = res
    return out

